# revision 26
# baseline (speedup 1.0000x reference)
"""Equivariant interaction block (gnn message passing) on 8 trn2 NeuronCores.

v2 strategy (vs v1 baseline at ~1.58 ms):
  * Host: sort edges by destination; core c owns node shard [c*npc, (c+1)*npc)
    and its incoming edges (zero-padded).  Host now ALSO precomputes every
    per-edge quantity derivable from inputs: the gathered source features in
    the three layouts the TP contractions need (af = [xs*sh_s, xv.sh_v],
    xs, xv transposed d-major), the per-tile 128x128 segment-selection
    matrices S, and folds 1/deg into the pass-2 combine weights.  This kills
    all pass-1 indirect DMAs, the on-device af build, and the dst==dst^T
    transpose chain.
  * Device pass 1 (per 128-edge tile):
      - radial MLP (PE) -> h2s bf16 [65, 512] per 512-edge supertile
      - W' = h2 @ w3p in PSUM (5 matmuls; col-permuted + CG-scaled on host)
      - ScalarE copies W' PSUM->SBUF bf16 in 2 chunks (hides behind PE work;
        PSUM pools stay single-buffered: 3+2+3 banks = 8)
      - DVE: block A mult at 2x (bf16 unit-stride, af broadcast on middle
        dim), fold-tree 48->24->12 at 2x, tensor_reduce [32,12];
        block C 3 d-multiplies at 2x + one 4D tensor_reduce; out_v assembly
      - GPSIMD: block B mult+reduce (otherwise idle)
      - PE: segment-combine matmul with host-provided S (bf16)
      - partials written fp32
  * Pass 2 (per 128-node block): two indirect gathers of combined rows,
    agg = p1*w0 + p2*w1 with 1/deg prefolded, PE transpose, fused irrep
    linear as two bf16 matmuls.
"""

import os
import sys

import numpy as np

for _p in ("/opt/trn_rl_repo", os.path.expanduser("~/.axon_site/_ro/trn_rl_repo")):
    if os.path.isdir(_p) and _p not in sys.path:
        sys.path.insert(0, _p)

import concourse.bacc as bacc
import concourse.bass as bass
import concourse.mybir as mybir
import concourse.tile as tile
from concourse.bass_utils import run_bass_kernel_spmd
import concourse.dve_ops as dve_ops
from concourse.dve_spec import Spec as DveSpec, Src0, Src1, AluOp as DAlu
from concourse.dve_spec import scan as dve_scan, lower as dve_lower
from concourse.dve_uop import DveOpSpec


def _register_mul_scan():
    """Custom DVE op: out = cumsum(in0 * in1) over the free dims (fp32
    accumulate).  Segment sums fall out as strided differences of the
    inclusive prefix sum, so one DVE pass replaces multiply + reduce."""
    name = "MUL_SCAN_ANT"
    for op in dve_ops.OPS:
        if op.name == name:
            return op

    def _ref(in0, in1, c0, c1, c2):
        b = (in0.astype(np.float32) * np.broadcast_to(in1, in0.shape))
        flat = b.reshape(b.shape[0], -1)
        return np.cumsum(flat, axis=1).reshape(in0.shape)

    spec = DveSpec(body=dve_scan(DAlu.ADD, Src0 * Src1), reference=_ref)
    op = dve_ops.DveOp(name, spec, subdim=False, uops_sha={})
    dve_ops.OPS.append(op)
    dve_ops._SUB_OPCODE_FOR_NAME[name] = (
        dve_ops._CUSTOM_DVE_ROW_BASE + len(dve_ops.OPS) - 1)
    dve_ops.CUSTOM_DVE_SPECS[name] = spec
    for ver in ("v3", "v4"):
        try:
            uops = dve_lower(spec, ver=ver)
            tmp = DveOpSpec(name=name, opcode=dve_ops.get_dve_sub_opcode(name),
                            uops=uops, rd1_en=True)
            op.uops_sha[ver] = tmp.sha(ver)
        except Exception:
            pass
    return op


MUL_SCAN = _register_mul_scan()

F32 = mybir.dt.float32
BF16 = mybir.dt.bfloat16
I32 = mybir.dt.int32
AF = mybir.ActivationFunctionType
OP = mybir.AluOpType

MUL0, MUL1 = 32, 16
RBF, HID = 16, 64
O1 = MUL0 * MUL0
O2 = O1 + MUL0 * MUL1
O3 = O2 + MUL1 * MUL1
WNUMEL = O3 + MUL1 * MUL0  # 2304
WCOLS = 2816  # device W' cols: A 1536 | B 512 | C 768 (d-replicated)
C_PATH = float(1.0 / np.sqrt(np.float32(MUL0 + MUL1)))
C_110 = float(1.0 / np.sqrt(3.0))
NCORES = 8
FDIM = MUL0 + 3 * MUL1  # 80
ECOLS = 260  # af 48 | xs 32 | xv3 48 | sh 4 | smat 128 (bf16 stream)

# use gpsimd for the block-B mult+reduce
GPSIMD_B = True


def _build_w3_perm():
    """Permutation + scale taking reference w3 columns into our layout.

    W' columns (2304):
      block A [0,1536):   q = o*48 + j      (o in 32, j in 48)
          j < 32 : path1  W1[i=j, o]   -> src i*32+o          scale C_PATH
          j >= 32: path4  W4[i=j-32,o] -> src O3 + i*32+o     scale C_PATH*C110
      block B [1536,2048): q = 1536 + o*32 + i (o in 16, i in 32)
          path2 W2[i, o] -> src O1 + i*16 + o                 scale C_PATH
      block C [2048,2304): q = 2048 + o*16 + i (o in 16, i in 16)
          path3 W3[i, o] -> src O2 + i*16 + o                 scale C_PATH
    """
    src = np.zeros(WCOLS, np.int64)
    scl = np.zeros(WCOLS, np.float32)
    for o in range(MUL0):
        for j in range(48):
            q = o * 48 + j
            if j < 32:
                src[q] = j * MUL0 + o
                scl[q] = C_PATH
            else:
                src[q] = O3 + (j - 32) * MUL0 + o
                scl[q] = C_PATH * C_110
    for o in range(MUL1):
        for i in range(MUL0):
            q = 1536 + o * 32 + i
            src[q] = O1 + i * MUL1 + o
            scl[q] = C_PATH
    for o in range(MUL1):
        for d in range(3):
            for i in range(MUL1):
                q = 2048 + o * 48 + d * 16 + i
                src[q] = O2 + i * MUL1 + o
                scl[q] = C_PATH
    return src, scl


def _irrep_matrix(ws, wv):
    """[80,80] M with out = in @ M equal to reference _irrep_linear."""
    M = np.zeros((FDIM, FDIM), np.float32)
    M[:MUL0, :MUL0] = ws
    for i in range(MUL1):
        for o in range(MUL1):
            for d in range(3):
                M[MUL0 + i * 3 + d, MUL0 + o * 3 + d] = wv[i, o]
    return M


def build_program(npc_pad, epad, num_cores, need=None):
    """Build + compile the SPMD bass program.

    need[b] = number of edge tiles that must be complete before node block b
    can run (pass-2 interleave schedule; max over cores).
    """
    t_sub = epad // 128
    t_super = epad // 512
    nb = npc_pad // 128
    if need is None:
        need = (t_sub,) * nb
    assert epad % 512 == 0 and npc_pad % 128 == 0

    nc = bacc.Bacc(
        "TRN2",
        target_bir_lowering=False,
        debug=False,
        enable_asserts=False,
        num_devices=num_cores,
    )

    xshT = nc.dram_tensor("xshT", [FDIM, npc_pad], BF16, kind="ExternalInput")
    ed2 = nc.dram_tensor("ed2", [epad, ECOLS], BF16, kind="ExternalInput")
    rbf17 = nc.dram_tensor("rbf17", [RBF + 1, epad], BF16, kind="ExternalInput")
    w1b = nc.dram_tensor("w1b", [RBF + 1, HID], BF16, kind="ExternalInput")
    w2b = nc.dram_tensor("w2b", [HID + 1, HID], BF16, kind="ExternalInput")
    w3p = nc.dram_tensor("w3p", [HID + 1, WCOLS], BF16, kind="ExternalInput")
    gw = nc.dram_tensor("gw", [npc_pad, 4], I32, kind="ExternalInput")
    msf = nc.dram_tensor("msf", [FDIM, FDIM], BF16, kind="ExternalInput")
    mout = nc.dram_tensor("mout", [FDIM, FDIM], BF16, kind="ExternalInput")
    ident = nc.dram_tensor("ident", [128, 128], F32, kind="ExternalInput")

    yT = nc.dram_tensor("yT", [FDIM, npc_pad], F32, kind="ExternalOutput")
    partials = nc.dram_tensor("partials", [epad, FDIM], F32)

    with tile.TileContext(nc) as tc:
        with (
            nc.allow_low_precision(reason="bf16 per-edge messages, fp32 aggregation"),
            tc.tile_pool(name="const", bufs=1) as cp,
            tc.tile_pool(name="sb", bufs=4) as sp,
            tc.tile_pool(name="sbs", bufs=3) as sps,
            tc.tile_pool(name="wa", bufs=1, space="PSUM") as wap,
            tc.tile_pool(name="wbc", bufs=1, space="PSUM") as wbcp,
            tc.tile_pool(name="pm", bufs=2, space="PSUM") as pmp,
        ):
            w1b_sb = cp.tile([RBF + 1, HID], BF16)
            nc.sync.dma_start(out=w1b_sb[:], in_=w1b[:])
            w2b_sb = cp.tile([HID + 1, HID], BF16)
            nc.sync.dma_start(out=w2b_sb[:], in_=w2b[:])
            w3p_sb = cp.tile([HID + 1, WCOLS], BF16)
            nc.sync.dma_start(out=w3p_sb[:], in_=w3p[:])
            ident_sb = cp.tile([128, 128], F32)
            nc.sync.dma_start(out=ident_sb[:], in_=ident[:])
            msf_sb = cp.tile([FDIM, FDIM], BF16)
            nc.sync.dma_start(out=msf_sb[:], in_=msf[:])
            mout_sb = cp.tile([FDIM, FDIM], BF16)
            nc.sync.dma_start(out=mout_sb[:], in_=mout[:])

            # ---------------- pass 2 body (interleaved) ----------------
            def node_block(b):
                n0, n1 = b * 128, (b + 1) * 128
                g_t = sp.tile([128, 4], I32, tag="g")
                nc.sync.dma_start(out=g_t[:], in_=gw[n0:n1, :])
                wv_t = g_t[:, 2:4].bitcast(F32)
                p1 = sp.tile([128, FDIM], F32, tag="p1")
                nc.gpsimd.indirect_dma_start(
                    out=p1[:], out_offset=None, in_=partials[:],
                    in_offset=bass.IndirectOffsetOnAxis(ap=g_t[:, 0:1], axis=0))
                p2 = sp.tile([128, FDIM], F32, tag="p2")
                nc.gpsimd.indirect_dma_start(
                    out=p2[:], out_offset=None, in_=partials[:],
                    in_offset=bass.IndirectOffsetOnAxis(ap=g_t[:, 1:2], axis=0))
                agg = sp.tile([128, FDIM], F32, tag="agg")
                nc.scalar.mul(agg[:], p1[:], wv_t[:, 0:1])
                nc.vector.scalar_tensor_tensor(
                    out=agg[:], in0=p2[:], scalar=wv_t[:, 1:2], in1=agg[:],
                    op0=OP.mult, op1=OP.add)

                tp_ps = pmp.tile([FDIM, 128], F32, tag="sm")
                nc.tensor.transpose(out=tp_ps[:], in_=agg[:],
                                    identity=ident_sb[:])
                aggnT = sp.tile([FDIM, 128], BF16, tag="aggnT")
                nc.scalar.copy(aggnT[:], tp_ps[:])

                xsh_t = sp.tile([FDIM, 128], BF16, tag="xsh")
                nc.sync.dma_start(out=xsh_t[:], in_=xshT[:, n0:n1])
                y_ps = pmp.tile([FDIM, 128], F32, tag="sm")
                nc.tensor.matmul(out=y_ps[:], lhsT=mout_sb[:], rhs=aggnT[:],
                                 start=True, stop=False)
                nc.tensor.matmul(out=y_ps[:], lhsT=msf_sb[:], rhs=xsh_t[:],
                                 start=False, stop=True)
                y_sb = sp.tile([FDIM, 128], F32, tag="y")
                nc.scalar.copy(y_sb[:], y_ps[:])
                nc.sync.dma_start(out=yT[:, n0:n1], in_=y_sb[:])

            next_b = [0]

            def emit_ready_blocks(tiles_done):
                while next_b[0] < nb and need[next_b[0]] <= tiles_done:
                    node_block(next_b[0])
                    next_b[0] += 1

            # ---------------- pass 1: edges ----------------
            def mlp(s):
                """radial MLP for supertile s -> h2s bf16 [65, 512]"""
                rbf_t = sps.tile([RBF + 1, 512], BF16, tag="rbf")
                nc.sync.dma_start(out=rbf_t[:], in_=rbf17[:, s * 512:(s + 1) * 512])

                h1_ps = pmp.tile([HID, 512], F32, tag="sm")
                nc.tensor.matmul(out=h1_ps[:], lhsT=w1b_sb[:], rhs=rbf_t[:],
                                 start=True, stop=True)
                h1s = sps.tile([HID + 1, 512], BF16, tag="h1s")
                nc.scalar.activation(h1s[:HID, :], h1_ps[:], AF.Silu)
                nc.gpsimd.memset(h1s[HID:HID + 1, :], 1.0)

                h2_ps = pmp.tile([HID, 512], F32, tag="sm")
                nc.tensor.matmul(out=h2_ps[:], lhsT=w2b_sb[:], rhs=h1s[:],
                                 start=True, stop=True)
                h2s = sps.tile([HID + 1, 512], BF16, tag="h2s")
                nc.scalar.activation(h2s[:HID, :], h2_ps[:], AF.Silu)
                nc.gpsimd.memset(h2s[HID:HID + 1, :], 1.0)
                return h2s

            h2s = mlp(0)
            for s in range(t_super):
                h2s_next = mlp(s + 1) if s + 1 < t_super else None

                for c in range(4):
                    t = s * 4 + c
                    r0, r1 = t * 128, (t + 1) * 128

                    ed_t = sp.tile([128, ECOLS], BF16, tag="ed")
                    nc.sync.dma_start(out=ed_t[:], in_=ed2[r0:r1, :])

                    af = ed_t[:, 0:48]
                    xs = ed_t[:, 48:80]
                    xv3 = ed_t[:, 80:128]
                    shs = ed_t[:, 128:129]
                    shv = ed_t[:, 129:132]
                    s_t = ed_t[:, 132:260]

                    scanA = sp.tile([128, 1600], F32, tag="scanA")
                    nc.gpsimd.memset(scanA[:, 0:1], 0.0)
                    scanC = sp.tile([128, 800], F32, tag="scanC")
                    nc.gpsimd.memset(scanC[:, 0:1], 0.0)

                    lhs = h2s[:, c * 128:(c + 1) * 128]
                    # W' matmuls into PSUM: A block 3x512, B+C 512+256
                    wa_ps = wap.tile([128, 1536], F32, tag="wa")
                    for u in range(3):
                        nc.tensor.matmul(
                            out=wa_ps[:, u * 512:(u + 1) * 512], lhsT=lhs,
                            rhs=w3p_sb[:, u * 512:(u + 1) * 512],
                            start=True, stop=True)
                    wbc_ps = wbcp.tile([128, 1280], F32, tag="wbc")
                    nc.tensor.matmul(out=wbc_ps[:, 0:512], lhsT=lhs,
                                     rhs=w3p_sb[:, 1536:2048],
                                     start=True, stop=True)
                    nc.tensor.matmul(out=wbc_ps[:, 512:1024], lhsT=lhs,
                                     rhs=w3p_sb[:, 2048:2560],
                                     start=True, stop=True)
                    nc.tensor.matmul(out=wbc_ps[:, 1024:1280], lhsT=lhs,
                                     rhs=w3p_sb[:, 2560:2816],
                                     start=True, stop=True)

                    # ScalarE: PSUM -> SBUF bf16 (overlaps with PE's next work)
                    wsb = sp.tile([128, WCOLS], BF16, tag="wsb")
                    nc.scalar.copy(wsb[:, 0:1536], wa_ps[:])
                    nc.scalar.copy(wsb[:, 1536:2816], wbc_ps[:])

                    m_t = sp.tile([128, FDIM], BF16, tag="m")

                    # ---- block B mult early on GPSIMD (result needed late) ----
                    wb_v = wsb[:, 1536:2048].rearrange("p (o i) -> p o i", i=32)
                    prodB = sp.tile([128, 512], BF16, tag="prodB")
                    pb_v = prodB[:].rearrange("p (o i) -> p o i", i=32)
                    t2 = sp.tile([128, 16], BF16, tag="t2")
                    nc.gpsimd.tensor_tensor(
                        out=pb_v, in0=wb_v,
                        in1=xs.unsqueeze(1).to_broadcast([128, 16, 32]),
                        op=OP.mult)

                    # ---- block A: out_s[o] = sum_j W'a[o,j] af[j] ----
                    # fused multiply+prefix-sum; segment sums = strided diffs
                    wa_v = wsb[:, 0:1536].rearrange("p (o j) -> p o j", j=48)
                    nc.vector._custom_dve(
                        MUL_SCAN,
                        out=scanA[:, 1:1537].rearrange("p (o j) -> p o j", j=48),
                        in0=wa_v,
                        in1=af.unsqueeze(1).to_broadcast([128, 32, 48]))
                    endA = scanA[:, 48:1584].rearrange("p (o j) -> p o j", j=48)
                    begA = scanA[:, 0:1536].rearrange("p (o j) -> p o j", j=48)
                    nc.vector.tensor_tensor(
                        out=m_t[:, 0:MUL0], in0=endA[:, :, 0],
                        in1=begA[:, :, 0], op=OP.subtract)

                    # ---- block B reduce ----
                    nc.vector.tensor_reduce(
                        out=t2[:], in_=pb_v, axis=mybir.AxisListType.X,
                        op=OP.add)

                    # ---- block C: V3[o,d] = sum_i W'c[o,d,i] xv[d,i] ----
                    nc.vector._custom_dve(
                        MUL_SCAN,
                        out=scanC[:, 1:769].rearrange("p (o k) -> p o k", k=48),
                        in0=wsb[:, 2048:2816].rearrange("p (o k) -> p o k", k=48),
                        in1=xv3.unsqueeze(1).to_broadcast([128, 16, 48]))
                    v3 = sp.tile([128, 48], BF16, tag="v3")
                    endC = scanC[:, 16:784].rearrange("p (s k) -> p s k", k=16)
                    begC = scanC[:, 0:768].rearrange("p (s k) -> p s k", k=16)
                    nc.vector.tensor_tensor(
                        out=v3[:], in0=endC[:, :, 0], in1=begC[:, :, 0],
                        op=OP.subtract)

                    # ---- out_v = t2 x sh_v + sh_s * V3 ----
                    tsh = sp.tile([128, 48], BF16, tag="tsh")
                    nc.gpsimd.tensor_tensor(
                        out=tsh[:].rearrange("p (o d) -> p o d", d=3),
                        in0=t2[:].unsqueeze(2).to_broadcast([128, 16, 3]),
                        in1=shv.unsqueeze(1).to_broadcast([128, 16, 3]),
                        op=OP.mult)
                    nc.vector.scalar_tensor_tensor(
                        out=m_t[:, MUL0:FDIM], in0=v3[:], scalar=shs,
                        in1=tsh[:], op0=OP.mult, op1=OP.add)

                    # ---- segment combine within tile: comb = S @ m ----
                    comb_ps = pmp.tile([128, FDIM], F32, tag="sm")
                    nc.tensor.matmul(out=comb_ps[:], lhsT=s_t[:], rhs=m_t[:],
                                     start=True, stop=True)
                    comb = sp.tile([128, FDIM], F32, tag="comb")
                    nc.scalar.copy(comb[:], comb_ps[:])
                    nc.sync.dma_start(out=partials[r0:r1, :], in_=comb[:])

                    emit_ready_blocks(t + 1)

                h2s = h2s_next

            emit_ready_blocks(t_sub)

    nc.compile()
    return nc


_PROGRAM_CACHE = {}


def _get_program(npc_pad, epad, num_cores, need):
    key = (npc_pad, epad, num_cores, need)
    if key not in _PROGRAM_CACHE:
        _PROGRAM_CACHE[key] = build_program(npc_pad, epad, num_cores, need)
    return _PROGRAM_CACHE[key]


def prepare_in_maps(x, edge_src, edge_dst, edge_sh, edge_rbf,
                    w1, b1, w2, b2, w3, b3, num_cores=NCORES):
    """Host-side sharding/layout prep. Returns (in_maps, meta)."""
    n = x.shape[0]
    npc = -(-n // num_cores)  # nodes per core
    npc_pad = -(-npc // 128) * 128

    dst = np.asarray(edge_dst, np.int64)
    src = np.asarray(edge_src, np.int64)
    order = np.argsort(dst, kind="stable")
    dst_s = dst[order]
    src_s = src[order]
    sh_s = np.asarray(edge_sh, np.float32)[order]
    rbf_s = np.asarray(edge_rbf, np.float32)[order]

    bounds = np.searchsorted(dst_s, np.arange(num_cores + 1) * npc)
    counts = np.diff(bounds)
    epad = max(512, int(-(-counts.max() // 512) * 512))

    bf16 = mybir.dt.np(BF16)
    w1b = np.concatenate([np.asarray(w1, np.float32),
                          np.asarray(b1, np.float32)[None, :]], 0).astype(bf16)
    w2b = np.concatenate([np.asarray(w2, np.float32),
                          np.asarray(b2, np.float32)[None, :]], 0).astype(bf16)
    perm, scl = _build_w3_perm()
    w3p_f = np.concatenate(
        [np.asarray(w3, np.float32)[:, perm] * scl[None, :],
         (np.asarray(b3, np.float32)[perm] * scl)[None, :]], 0)
    w3p = w3p_f.astype(bf16)
    ident = np.eye(128, dtype=np.float32)
    xf = np.asarray(x, np.float32)

    in_maps = []
    meta = {"npc": npc, "npc_pad": npc_pad, "epad": epad, "n": n,
            "num_cores": num_cores}
    for c in range(num_cores):
        lo, hi = bounds[c], bounds[c + 1]
        ec = hi - lo
        csrc = src_s[lo:hi]
        cdst = dst_s[lo:hi]
        csh = sh_s[lo:hi]  # [ec, 4]

        # per-edge feature stream [epad, ECOLS] bf16
        xg = xf[csrc]                        # [ec, 80]
        xs = xg[:, :MUL0]                    # [ec, 32]
        xv = xg[:, MUL0:].reshape(ec, MUL1, 3)
        ed = np.zeros((epad, ECOLS), np.float32)
        ed[:ec, 0:32] = xs * csh[:, 0:1]
        ed[:ec, 32:48] = np.einsum('eid,ed->ei', xv, csh[:, 1:4])
        ed[:ec, 48:80] = xs
        ed[:ec, 80:128] = xv.transpose(0, 2, 1).reshape(ec, 48)
        ed[:ec, 128:132] = csh
        # per-tile selection matrices in cols 132:260
        ntile = epad // 128
        dpad = np.full(epad, -1, np.int64)
        dpad[:ec] = cdst
        dt = dpad.reshape(ntile, 128)
        S = (dt[:, :, None] == dt[:, None, :]).astype(np.float32)
        ed[:, 132:260] = S.reshape(epad, 128)
        ced = ed.astype(bf16)

        crbf = np.zeros((RBF + 1, epad), np.float32)
        crbf[:RBF, :ec] = rbf_s[lo:hi].T
        crbf[RBF, :] = 1.0
        crbf = crbf.astype(bf16)

        # node -> first/last edge rows (local), weights with 1/deg folded
        nbase = c * npc
        nodes = np.arange(npc_pad, dtype=np.int64) + nbase
        first = np.searchsorted(cdst, nodes, side="left")
        last = np.searchsorted(cdst, nodes, side="right") - 1
        deg = (last - first + 1).astype(np.int64)
        has = deg > 0
        gwbuf = np.zeros((npc_pad, 4), np.int32)
        wv = np.zeros((npc_pad, 2), np.float32)
        gwbuf[has, 0] = first[has].astype(np.int32)
        gwbuf[has, 1] = last[has].astype(np.int32)
        inv = 1.0 / np.maximum(deg, 1).astype(np.float32)
        wv[has, 0] = inv[has]
        wv[has, 1] = (((first[has] // 128) != (last[has] // 128))
                      .astype(np.float32) * inv[has])
        gwbuf[:, 2:4] = wv.view(np.int32)

        # pass-2 interleave: edge tiles needed before node block b can run
        lastpad = np.where(has, last, 0)
        nblk = npc_pad // 128
        blk_last = lastpad.reshape(nblk, 128).max(axis=1)
        cneed = (blk_last // 128 + 1).astype(np.int64)
        meta.setdefault("need", []).append(cneed)

        cxsh = np.zeros((FDIM, npc_pad), np.float32)
        sl = xf[nbase:min(nbase + npc, n)]
        cxsh[:, :sl.shape[0]] = sl.T

        in_maps.append({
            "xshT": cxsh.astype(bf16), "ed2": ced,
            "rbf17": crbf, "w1b": w1b, "w2b": w2b, "w3p": w3p,
            "gw": gwbuf, "ident": ident,
        })
    return in_maps, meta


def kernel(x, edge_src, edge_dst, edge_sh, edge_rbf,
           w1, b1, w2, b2, w3, b3, ws_self, wv_self, ws_out, wv_out,
           _trace=False):
    num_cores = NCORES
    in_maps, meta = prepare_in_maps(
        x, edge_src, edge_dst, edge_sh, edge_rbf, w1, b1, w2, b2, w3, b3,
        num_cores=num_cores)
    bf16 = mybir.dt.np(BF16)
    msf = _irrep_matrix(np.asarray(ws_self, np.float32),
                        np.asarray(wv_self, np.float32)).astype(bf16)
    mout = _irrep_matrix(np.asarray(ws_out, np.float32),
                         np.asarray(wv_out, np.float32)).astype(bf16)
    for m in in_maps:
        m["msf"] = msf
        m["mout"] = mout

    need = tuple(int(v) for v in np.max(np.stack(meta["need"]), axis=0))
    nc = _get_program(meta["npc_pad"], meta["epad"], num_cores, need)
    res = run_bass_kernel_spmd(nc, in_maps, list(range(num_cores)),
                               trace=_trace)

    n, npc = meta["n"], meta["npc"]
    y = np.empty((n, FDIM), np.float32)
    for c in range(num_cores):
        lo = c * npc
        hi = min(lo + npc, n)
        y[lo:hi] = np.asarray(res.results[c]["yT"])[:, :hi - lo].T
    kernel._last_results = res
    return y


# revision 28
# speedup vs baseline: 1.0415x; 1.0415x over previous
"""Equivariant interaction block (gnn message passing) on 8 trn2 NeuronCores.

v2 strategy (vs v1 baseline at ~1.58 ms):
  * Host: sort edges by destination; core c owns node shard [c*npc, (c+1)*npc)
    and its incoming edges (zero-padded).  Host now ALSO precomputes every
    per-edge quantity derivable from inputs: the gathered source features in
    the three layouts the TP contractions need (af = [xs*sh_s, xv.sh_v],
    xs, xv transposed d-major), the per-tile 128x128 segment-selection
    matrices S, and folds 1/deg into the pass-2 combine weights.  This kills
    all pass-1 indirect DMAs, the on-device af build, and the dst==dst^T
    transpose chain.
  * Device pass 1 (per 128-edge tile):
      - radial MLP (PE) -> h2s bf16 [65, 512] per 512-edge supertile
      - W' = h2 @ w3p in PSUM (5 matmuls; col-permuted + CG-scaled on host)
      - ScalarE copies W' PSUM->SBUF bf16 in 2 chunks (hides behind PE work;
        PSUM pools stay single-buffered: 3+2+3 banks = 8)
      - DVE: block A mult at 2x (bf16 unit-stride, af broadcast on middle
        dim), fold-tree 48->24->12 at 2x, tensor_reduce [32,12];
        block C 3 d-multiplies at 2x + one 4D tensor_reduce; out_v assembly
      - GPSIMD: block B mult+reduce (otherwise idle)
      - PE: segment-combine matmul with host-provided S (bf16)
      - partials written fp32
  * Pass 2 (per 128-node block): two indirect gathers of combined rows,
    agg = p1*w0 + p2*w1 with 1/deg prefolded, PE transpose, fused irrep
    linear as two bf16 matmuls.
"""

import os
import sys

import numpy as np

for _p in ("/opt/trn_rl_repo", os.path.expanduser("~/.axon_site/_ro/trn_rl_repo")):
    if os.path.isdir(_p) and _p not in sys.path:
        sys.path.insert(0, _p)

import concourse.bacc as bacc
import concourse.bass as bass
import concourse.mybir as mybir
import concourse.tile as tile
from concourse.bass_utils import run_bass_kernel_spmd
import concourse.dve_ops as dve_ops
from concourse.dve_spec import Spec as DveSpec, Src0, Src1, AluOp as DAlu
from concourse.dve_spec import scan as dve_scan, lower as dve_lower
from concourse.dve_uop import DveOpSpec


def _register_mul_scan():
    """Custom DVE op: out = cumsum(in0 * in1) over the free dims (fp32
    accumulate).  Segment sums fall out as strided differences of the
    inclusive prefix sum, so one DVE pass replaces multiply + reduce."""
    name = "MUL_SCAN_ANT"
    for op in dve_ops.OPS:
        if op.name == name:
            return op

    def _ref(in0, in1, c0, c1, c2):
        b = (in0.astype(np.float32) * np.broadcast_to(in1, in0.shape))
        flat = b.reshape(b.shape[0], -1)
        return np.cumsum(flat, axis=1).reshape(in0.shape)

    spec = DveSpec(body=dve_scan(DAlu.ADD, Src0 * Src1), reference=_ref)
    op = dve_ops.DveOp(name, spec, subdim=False, uops_sha={})
    dve_ops.OPS.append(op)
    dve_ops._SUB_OPCODE_FOR_NAME[name] = (
        dve_ops._CUSTOM_DVE_ROW_BASE + len(dve_ops.OPS) - 1)
    dve_ops.CUSTOM_DVE_SPECS[name] = spec
    for ver in ("v3", "v4"):
        try:
            uops = dve_lower(spec, ver=ver)
            tmp = DveOpSpec(name=name, opcode=dve_ops.get_dve_sub_opcode(name),
                            uops=uops, rd1_en=True)
            op.uops_sha[ver] = tmp.sha(ver)
        except Exception:
            pass
    return op


MUL_SCAN = _register_mul_scan()

F32 = mybir.dt.float32
BF16 = mybir.dt.bfloat16
I32 = mybir.dt.int32
AF = mybir.ActivationFunctionType
OP = mybir.AluOpType

MUL0, MUL1 = 32, 16
RBF, HID = 16, 64
O1 = MUL0 * MUL0
O2 = O1 + MUL0 * MUL1
O3 = O2 + MUL1 * MUL1
WNUMEL = O3 + MUL1 * MUL0  # 2304
WCOLS = 2816  # device W' cols: A 1536 | B 512 | C 768 (d-replicated)
C_PATH = float(1.0 / np.sqrt(np.float32(MUL0 + MUL1)))
C_110 = float(1.0 / np.sqrt(3.0))
NCORES = 8
FDIM = MUL0 + 3 * MUL1  # 80
ECOLS = 260  # af 48 | xs 32 | xv3 48 | sh 4 | smat 128 (bf16 stream)

# use gpsimd for the block-B mult+reduce
GPSIMD_B = True


def _build_w3_perm():
    """Permutation + scale taking reference w3 columns into our layout.

    W' columns (2304):
      block A [0,1536):   q = o*48 + j      (o in 32, j in 48)
          j < 32 : path1  W1[i=j, o]   -> src i*32+o          scale C_PATH
          j >= 32: path4  W4[i=j-32,o] -> src O3 + i*32+o     scale C_PATH*C110
      block B [1536,2048): q = 1536 + o*32 + i (o in 16, i in 32)
          path2 W2[i, o] -> src O1 + i*16 + o                 scale C_PATH
      block C [2048,2304): q = 2048 + o*16 + i (o in 16, i in 16)
          path3 W3[i, o] -> src O2 + i*16 + o                 scale C_PATH
    """
    src = np.zeros(WCOLS, np.int64)
    scl = np.zeros(WCOLS, np.float32)
    for o in range(MUL0):
        for j in range(48):
            q = o * 48 + j
            if j < 32:
                src[q] = j * MUL0 + o
                scl[q] = C_PATH
            else:
                src[q] = O3 + (j - 32) * MUL0 + o
                scl[q] = C_PATH * C_110
    for o in range(MUL1):
        for i in range(MUL0):
            q = 1536 + o * 32 + i
            src[q] = O1 + i * MUL1 + o
            scl[q] = C_PATH
    for o in range(MUL1):
        for d in range(3):
            for i in range(MUL1):
                q = 2048 + o * 48 + d * 16 + i
                src[q] = O2 + i * MUL1 + o
                scl[q] = C_PATH
    return src, scl


def _irrep_matrix(ws, wv):
    """[80,80] M with out = in @ M equal to reference _irrep_linear."""
    M = np.zeros((FDIM, FDIM), np.float32)
    M[:MUL0, :MUL0] = ws
    for i in range(MUL1):
        for o in range(MUL1):
            for d in range(3):
                M[MUL0 + i * 3 + d, MUL0 + o * 3 + d] = wv[i, o]
    return M


def build_program(npc_pad, epad, num_cores, need=None):
    """Build + compile the SPMD bass program.

    need[b] = number of edge tiles that must be complete before node block b
    can run (pass-2 interleave schedule; max over cores).
    """
    t_sub = epad // 128
    t_super = epad // 512
    nb = npc_pad // 128
    if need is None:
        need = (t_sub,) * nb
    assert epad % 512 == 0 and npc_pad % 128 == 0

    nc = bacc.Bacc(
        "TRN2",
        target_bir_lowering=False,
        debug=False,
        enable_asserts=False,
        num_devices=num_cores,
    )

    xshT = nc.dram_tensor("xshT", [FDIM, npc_pad], BF16, kind="ExternalInput")
    ed2 = nc.dram_tensor("ed2", [epad, ECOLS], BF16, kind="ExternalInput")
    rbf17 = nc.dram_tensor("rbf17", [RBF + 1, epad], BF16, kind="ExternalInput")
    w1b = nc.dram_tensor("w1b", [RBF + 1, HID], BF16, kind="ExternalInput")
    w2b = nc.dram_tensor("w2b", [HID + 1, HID], BF16, kind="ExternalInput")
    w3p = nc.dram_tensor("w3p", [HID + 1, WCOLS], BF16, kind="ExternalInput")
    gw = nc.dram_tensor("gw", [npc_pad, 4], I32, kind="ExternalInput")
    msf = nc.dram_tensor("msf", [FDIM, FDIM], BF16, kind="ExternalInput")
    mout = nc.dram_tensor("mout", [FDIM, FDIM], BF16, kind="ExternalInput")
    ident = nc.dram_tensor("ident", [128, 128], F32, kind="ExternalInput")

    yT = nc.dram_tensor("yT", [FDIM, npc_pad], F32, kind="ExternalOutput")
    partials = nc.dram_tensor("partials", [epad, FDIM], F32)

    with tile.TileContext(nc) as tc:
        with (
            nc.allow_low_precision(reason="bf16 per-edge messages, fp32 aggregation"),
            tc.tile_pool(name="const", bufs=1) as cp,
            tc.tile_pool(name="sb", bufs=4) as sp,
            tc.tile_pool(name="sbs", bufs=3) as sps,
            tc.tile_pool(name="wa", bufs=1, space="PSUM") as wap,
            tc.tile_pool(name="wbc", bufs=1, space="PSUM") as wbcp,
            tc.tile_pool(name="pm", bufs=2, space="PSUM") as pmp,
        ):
            w1b_sb = cp.tile([RBF + 1, HID], BF16)
            nc.sync.dma_start(out=w1b_sb[:], in_=w1b[:])
            w2b_sb = cp.tile([HID + 1, HID], BF16)
            nc.sync.dma_start(out=w2b_sb[:], in_=w2b[:])
            w3p_sb = cp.tile([HID + 1, WCOLS], BF16)
            nc.sync.dma_start(out=w3p_sb[:], in_=w3p[:])
            ident_sb = cp.tile([128, 128], F32)
            nc.sync.dma_start(out=ident_sb[:], in_=ident[:])
            msf_sb = cp.tile([FDIM, FDIM], BF16)
            nc.sync.dma_start(out=msf_sb[:], in_=msf[:])
            mout_sb = cp.tile([FDIM, FDIM], BF16)
            nc.sync.dma_start(out=mout_sb[:], in_=mout[:])

            # ---------------- pass 2 body (interleaved) ----------------
            def node_block(b):
                n0, n1 = b * 128, (b + 1) * 128
                g_t = sp.tile([128, 4], I32, tag="g")
                nc.sync.dma_start(out=g_t[:], in_=gw[n0:n1, :])
                wv_t = g_t[:, 2:4].bitcast(F32)
                p1 = sp.tile([128, FDIM], F32, tag="p1")
                nc.gpsimd.indirect_dma_start(
                    out=p1[:], out_offset=None, in_=partials[:],
                    in_offset=bass.IndirectOffsetOnAxis(ap=g_t[:, 0:1], axis=0))
                p2 = sp.tile([128, FDIM], F32, tag="p2")
                nc.gpsimd.indirect_dma_start(
                    out=p2[:], out_offset=None, in_=partials[:],
                    in_offset=bass.IndirectOffsetOnAxis(ap=g_t[:, 1:2], axis=0))
                agg = sp.tile([128, FDIM], F32, tag="agg")
                nc.vector.tensor_scalar(
                    out=agg[:], in0=p1[:], scalar1=wv_t[:, 0:1], scalar2=None,
                    op0=OP.mult)
                nc.vector.scalar_tensor_tensor(
                    out=agg[:], in0=p2[:], scalar=wv_t[:, 1:2], in1=agg[:],
                    op0=OP.mult, op1=OP.add)

                tp_ps = pmp.tile([FDIM, 128], F32, tag="sm")
                nc.tensor.transpose(out=tp_ps[:], in_=agg[:],
                                    identity=ident_sb[:])
                aggnT = sp.tile([FDIM, 128], BF16, tag="aggnT")
                nc.scalar.copy(aggnT[:], tp_ps[:])

                xsh_t = sp.tile([FDIM, 128], BF16, tag="xsh")
                nc.sync.dma_start(out=xsh_t[:], in_=xshT[:, n0:n1])
                y_ps = pmp.tile([FDIM, 128], F32, tag="sm")
                nc.tensor.matmul(out=y_ps[:], lhsT=mout_sb[:], rhs=aggnT[:],
                                 start=True, stop=False)
                nc.tensor.matmul(out=y_ps[:], lhsT=msf_sb[:], rhs=xsh_t[:],
                                 start=False, stop=True)
                y_sb = sp.tile([FDIM, 128], F32, tag="y")
                nc.scalar.copy(y_sb[:], y_ps[:])
                nc.sync.dma_start(out=yT[:, n0:n1], in_=y_sb[:])

            next_b = [0]

            def emit_ready_blocks(tiles_done):
                while next_b[0] < nb and need[next_b[0]] <= tiles_done:
                    node_block(next_b[0])
                    next_b[0] += 1

            # ---------------- pass 1: edges ----------------
            def mlp(s):
                """radial MLP for supertile s -> h2s bf16 [65, 512]"""
                rbf_t = sps.tile([RBF + 1, 512], BF16, tag="rbf")
                nc.sync.dma_start(out=rbf_t[:], in_=rbf17[:, s * 512:(s + 1) * 512])

                h1_ps = pmp.tile([HID, 512], F32, tag="sm")
                nc.tensor.matmul(out=h1_ps[:], lhsT=w1b_sb[:], rhs=rbf_t[:],
                                 start=True, stop=True)
                h1s = sps.tile([HID + 1, 512], BF16, tag="h1s")
                nc.scalar.activation(h1s[:HID, :], h1_ps[:], AF.Silu)
                nc.gpsimd.memset(h1s[HID:HID + 1, :], 1.0)

                h2_ps = pmp.tile([HID, 512], F32, tag="sm")
                nc.tensor.matmul(out=h2_ps[:], lhsT=w2b_sb[:], rhs=h1s[:],
                                 start=True, stop=True)
                h2s = sps.tile([HID + 1, 512], BF16, tag="h2s")
                nc.scalar.activation(h2s[:HID, :], h2_ps[:], AF.Silu)
                nc.gpsimd.memset(h2s[HID:HID + 1, :], 1.0)
                return h2s

            h2s = mlp(0)
            for s in range(t_super):
                h2s_next = mlp(s + 1) if s + 1 < t_super else None

                for c in range(4):
                    t = s * 4 + c
                    r0, r1 = t * 128, (t + 1) * 128

                    ed_t = sp.tile([128, ECOLS], BF16, tag="ed")
                    nc.sync.dma_start(out=ed_t[:], in_=ed2[r0:r1, :])

                    af = ed_t[:, 0:48]
                    xs = ed_t[:, 48:80]
                    xv3 = ed_t[:, 80:128]
                    shs = ed_t[:, 128:129]
                    shv = ed_t[:, 129:132]
                    s_t = ed_t[:, 132:260]

                    scanA = sp.tile([128, 1600], F32, tag="scanA")
                    nc.gpsimd.memset(scanA[:, 0:1], 0.0)
                    scanC = sp.tile([128, 800], F32, tag="scanC")
                    nc.gpsimd.memset(scanC[:, 0:1], 0.0)

                    lhs = h2s[:, c * 128:(c + 1) * 128]
                    # W' matmuls into PSUM: A block 3x512, B+C 512+256
                    wa_ps = wap.tile([128, 1536], F32, tag="wa")
                    for u in range(3):
                        nc.tensor.matmul(
                            out=wa_ps[:, u * 512:(u + 1) * 512], lhsT=lhs,
                            rhs=w3p_sb[:, u * 512:(u + 1) * 512],
                            start=True, stop=True)
                    wbc_ps = wbcp.tile([128, 1280], F32, tag="wbc")
                    nc.tensor.matmul(out=wbc_ps[:, 0:512], lhsT=lhs,
                                     rhs=w3p_sb[:, 1536:2048],
                                     start=True, stop=True)
                    nc.tensor.matmul(out=wbc_ps[:, 512:1024], lhsT=lhs,
                                     rhs=w3p_sb[:, 2048:2560],
                                     start=True, stop=True)
                    nc.tensor.matmul(out=wbc_ps[:, 1024:1280], lhsT=lhs,
                                     rhs=w3p_sb[:, 2560:2816],
                                     start=True, stop=True)

                    # ScalarE: PSUM -> SBUF bf16 (overlaps with PE's next work)
                    wsb = sp.tile([128, WCOLS], BF16, tag="wsb")
                    nc.scalar.copy(wsb[:, 0:1536], wa_ps[:])
                    nc.scalar.copy(wsb[:, 1536:2816], wbc_ps[:])

                    m_t = sp.tile([128, FDIM], BF16, tag="m")

                    # ---- block B mult early on GPSIMD (result needed late) ----
                    wb_v = wsb[:, 1536:2048].rearrange("p (o i) -> p o i", i=32)
                    prodB = sp.tile([128, 512], BF16, tag="prodB")
                    pb_v = prodB[:].rearrange("p (o i) -> p o i", i=32)
                    t2 = sp.tile([128, 16], BF16, tag="t2")
                    nc.gpsimd.tensor_tensor(
                        out=pb_v, in0=wb_v,
                        in1=xs.unsqueeze(1).to_broadcast([128, 16, 32]),
                        op=OP.mult)

                    # ---- block A: out_s[o] = sum_j W'a[o,j] af[j] ----
                    # fused multiply+prefix-sum; segment sums = strided diffs
                    wa_v = wsb[:, 0:1536].rearrange("p (o j) -> p o j", j=48)
                    nc.vector._custom_dve(
                        MUL_SCAN,
                        out=scanA[:, 1:1537].rearrange("p (o j) -> p o j", j=48),
                        in0=wa_v,
                        in1=af.unsqueeze(1).to_broadcast([128, 32, 48]))
                    endA = scanA[:, 48:1584].rearrange("p (o j) -> p o j", j=48)
                    begA = scanA[:, 0:1536].rearrange("p (o j) -> p o j", j=48)
                    nc.vector.tensor_tensor(
                        out=m_t[:, 0:MUL0], in0=endA[:, :, 0],
                        in1=begA[:, :, 0], op=OP.subtract)

                    # ---- block B reduce ----
                    nc.vector.tensor_reduce(
                        out=t2[:], in_=pb_v, axis=mybir.AxisListType.X,
                        op=OP.add)

                    # ---- block C: V3[o,d] = sum_i W'c[o,d,i] xv[d,i] ----
                    nc.vector._custom_dve(
                        MUL_SCAN,
                        out=scanC[:, 1:769].rearrange("p (o k) -> p o k", k=48),
                        in0=wsb[:, 2048:2816].rearrange("p (o k) -> p o k", k=48),
                        in1=xv3.unsqueeze(1).to_broadcast([128, 16, 48]))
                    v3 = sp.tile([128, 48], BF16, tag="v3")
                    endC = scanC[:, 16:784].rearrange("p (s k) -> p s k", k=16)
                    begC = scanC[:, 0:768].rearrange("p (s k) -> p s k", k=16)
                    nc.vector.tensor_tensor(
                        out=v3[:], in0=endC[:, :, 0], in1=begC[:, :, 0],
                        op=OP.subtract)

                    # ---- out_v = t2 x sh_v + sh_s * V3 ----
                    tsh = sp.tile([128, 48], BF16, tag="tsh")
                    nc.vector.tensor_tensor(
                        out=tsh[:].rearrange("p (o d) -> p o d", d=3),
                        in0=t2[:].unsqueeze(2).to_broadcast([128, 16, 3]),
                        in1=shv.unsqueeze(1).to_broadcast([128, 16, 3]),
                        op=OP.mult)
                    nc.vector.scalar_tensor_tensor(
                        out=m_t[:, MUL0:FDIM], in0=v3[:], scalar=shs,
                        in1=tsh[:], op0=OP.mult, op1=OP.add)

                    # ---- segment combine within tile: comb = S @ m ----
                    comb_ps = pmp.tile([128, FDIM], F32, tag="sm")
                    nc.tensor.matmul(out=comb_ps[:], lhsT=s_t[:], rhs=m_t[:],
                                     start=True, stop=True)
                    comb = sp.tile([128, FDIM], F32, tag="comb")
                    nc.scalar.copy(comb[:], comb_ps[:])
                    nc.sync.dma_start(out=partials[r0:r1, :], in_=comb[:])

                    emit_ready_blocks(t + 1)

                h2s = h2s_next

            emit_ready_blocks(t_sub)

    nc.compile()
    return nc


_PROGRAM_CACHE = {}


def _get_program(npc_pad, epad, num_cores, need):
    key = (npc_pad, epad, num_cores, need)
    if key not in _PROGRAM_CACHE:
        _PROGRAM_CACHE[key] = build_program(npc_pad, epad, num_cores, need)
    return _PROGRAM_CACHE[key]


def prepare_in_maps(x, edge_src, edge_dst, edge_sh, edge_rbf,
                    w1, b1, w2, b2, w3, b3, num_cores=NCORES):
    """Host-side sharding/layout prep. Returns (in_maps, meta)."""
    n = x.shape[0]
    npc = -(-n // num_cores)  # nodes per core
    npc_pad = -(-npc // 128) * 128

    dst = np.asarray(edge_dst, np.int64)
    src = np.asarray(edge_src, np.int64)
    order = np.argsort(dst, kind="stable")
    dst_s = dst[order]
    src_s = src[order]
    sh_s = np.asarray(edge_sh, np.float32)[order]
    rbf_s = np.asarray(edge_rbf, np.float32)[order]

    bounds = np.searchsorted(dst_s, np.arange(num_cores + 1) * npc)
    counts = np.diff(bounds)
    epad = max(512, int(-(-counts.max() // 512) * 512))

    bf16 = mybir.dt.np(BF16)
    w1b = np.concatenate([np.asarray(w1, np.float32),
                          np.asarray(b1, np.float32)[None, :]], 0).astype(bf16)
    w2b = np.concatenate([np.asarray(w2, np.float32),
                          np.asarray(b2, np.float32)[None, :]], 0).astype(bf16)
    perm, scl = _build_w3_perm()
    w3p_f = np.concatenate(
        [np.asarray(w3, np.float32)[:, perm] * scl[None, :],
         (np.asarray(b3, np.float32)[perm] * scl)[None, :]], 0)
    w3p = w3p_f.astype(bf16)
    ident = np.eye(128, dtype=np.float32)
    xf = np.asarray(x, np.float32)

    in_maps = []
    meta = {"npc": npc, "npc_pad": npc_pad, "epad": epad, "n": n,
            "num_cores": num_cores}
    for c in range(num_cores):
        lo, hi = bounds[c], bounds[c + 1]
        ec = hi - lo
        csrc = src_s[lo:hi]
        cdst = dst_s[lo:hi]
        csh = sh_s[lo:hi]  # [ec, 4]

        # per-edge feature stream [epad, ECOLS] bf16
        xg = xf[csrc]                        # [ec, 80]
        xs = xg[:, :MUL0]                    # [ec, 32]
        xv = xg[:, MUL0:].reshape(ec, MUL1, 3)
        ed = np.zeros((epad, ECOLS), np.float32)
        ed[:ec, 0:32] = xs * csh[:, 0:1]
        ed[:ec, 32:48] = np.einsum('eid,ed->ei', xv, csh[:, 1:4])
        ed[:ec, 48:80] = xs
        ed[:ec, 80:128] = xv.transpose(0, 2, 1).reshape(ec, 48)
        ed[:ec, 128:132] = csh
        # per-tile selection matrices in cols 132:260
        ntile = epad // 128
        dpad = np.full(epad, -1, np.int64)
        dpad[:ec] = cdst
        dt = dpad.reshape(ntile, 128)
        S = (dt[:, :, None] == dt[:, None, :]).astype(np.float32)
        ed[:, 132:260] = S.reshape(epad, 128)
        ced = ed.astype(bf16)

        crbf = np.zeros((RBF + 1, epad), np.float32)
        crbf[:RBF, :ec] = rbf_s[lo:hi].T
        crbf[RBF, :] = 1.0
        crbf = crbf.astype(bf16)

        # node -> first/last edge rows (local), weights with 1/deg folded
        nbase = c * npc
        nodes = np.arange(npc_pad, dtype=np.int64) + nbase
        first = np.searchsorted(cdst, nodes, side="left")
        last = np.searchsorted(cdst, nodes, side="right") - 1
        deg = (last - first + 1).astype(np.int64)
        has = deg > 0
        gwbuf = np.zeros((npc_pad, 4), np.int32)
        wv = np.zeros((npc_pad, 2), np.float32)
        gwbuf[has, 0] = first[has].astype(np.int32)
        gwbuf[has, 1] = last[has].astype(np.int32)
        inv = 1.0 / np.maximum(deg, 1).astype(np.float32)
        wv[has, 0] = inv[has]
        wv[has, 1] = (((first[has] // 128) != (last[has] // 128))
                      .astype(np.float32) * inv[has])
        gwbuf[:, 2:4] = wv.view(np.int32)

        # pass-2 interleave: edge tiles needed before node block b can run
        lastpad = np.where(has, last, 0)
        nblk = npc_pad // 128
        blk_last = lastpad.reshape(nblk, 128).max(axis=1)
        cneed = (blk_last // 128 + 1).astype(np.int64)
        meta.setdefault("need", []).append(cneed)

        cxsh = np.zeros((FDIM, npc_pad), np.float32)
        sl = xf[nbase:min(nbase + npc, n)]
        cxsh[:, :sl.shape[0]] = sl.T

        in_maps.append({
            "xshT": cxsh.astype(bf16), "ed2": ced,
            "rbf17": crbf, "w1b": w1b, "w2b": w2b, "w3p": w3p,
            "gw": gwbuf, "ident": ident,
        })
    return in_maps, meta


def kernel(x, edge_src, edge_dst, edge_sh, edge_rbf,
           w1, b1, w2, b2, w3, b3, ws_self, wv_self, ws_out, wv_out,
           _trace=False):
    num_cores = NCORES
    in_maps, meta = prepare_in_maps(
        x, edge_src, edge_dst, edge_sh, edge_rbf, w1, b1, w2, b2, w3, b3,
        num_cores=num_cores)
    bf16 = mybir.dt.np(BF16)
    msf = _irrep_matrix(np.asarray(ws_self, np.float32),
                        np.asarray(wv_self, np.float32)).astype(bf16)
    mout = _irrep_matrix(np.asarray(ws_out, np.float32),
                         np.asarray(wv_out, np.float32)).astype(bf16)
    for m in in_maps:
        m["msf"] = msf
        m["mout"] = mout

    need = tuple(int(v) for v in np.max(np.stack(meta["need"]), axis=0))
    nc = _get_program(meta["npc_pad"], meta["epad"], num_cores, need)
    res = run_bass_kernel_spmd(nc, in_maps, list(range(num_cores)),
                               trace=_trace)

    n, npc = meta["n"], meta["npc"]
    y = np.empty((n, FDIM), np.float32)
    for c in range(num_cores):
        lo = c * npc
        hi = min(lo + npc, n)
        y[lo:hi] = np.asarray(res.results[c]["yT"])[:, :hi - lo].T
    kernel._last_results = res
    return y
